# revision 5
# baseline (speedup 1.0000x reference)
"""Trainium2 Bass kernel for CausalSE (chunked-mean-pool -> per-channel EMA ->
int4-fake-quant SE bottleneck -> sigmoid gate -> gated residual).

Contract: kernel(**inputs) takes FULL unsharded inputs (as produced by
setup_inputs) and returns the FULL [16, 2048, 4096] float32 output.
Internally shards batch 16 -> 8 NeuronCores (2 per core), replicating the
small weights, and runs a single-pass streaming Bass/Tile kernel per core.

Algorithm notes:
  - pooled mean and the (1-r) EMA input scale are folded into the first SE
    matmul weights: scan computes q[t] = r*q[t-1] + chunk_sum[t], and
    W1' = fq(w1) * ((1-r)/16) per input channel, so h = s*q never needs to be
    materialized.
  - EMA runs as one hardware TensorTensorScan per (batch, time-block) over the
    flattened (channel-block, pooled-t) axis; r is masked to 0 at each
    channel-block's first pooled step so segments don't leak, and the carry
    from the previous time block is injected into the first chunk-sum.
  - Weight fake-quant (int4 symmetric, round-half-even) is exact host-side
    preprocessing of tiny tensors; all x-dependent compute runs on device.
"""

import contextlib

import numpy as np

import concourse.bacc as bacc
import concourse.mybir as mybir
import concourse.tile as tile
from concourse import bass_utils

F32 = mybir.dt.float32

B = 16
C = 2048
L = 4096
CHUNK = 16
HID = 256
QMAX = 7
EPS = 1e-5
N_CORES = 8
BPC = B // N_CORES          # batches per core = 2
P = 128
NCB = C // P                # channel blocks = 16
NOC = HID // P              # hidden (SE bottleneck) blocks = 2
TBLK = 512                  # time elements per streamed block
NTB = L // TBLK             # time blocks = 8
TP = TBLK // CHUNK          # pooled steps per block = 32

_CACHE = {}


def _emit_body(nc, xd, outd, w1, w2, b1, b2, rmask, rlast,
               xpool, spool, carrypool, ps1, ps2):
    """One full pass over this core's two batch elements."""
    for b in range(BPC):
        qc = carrypool.tile([P, NCB], F32, tag=f"qc{b}")
        for k in range(NTB):
            t0 = k * TBLK
            xt = xpool.tile([P, NCB, TBLK], F32, tag="xt")
            nc.sync.dma_start(
                xt[:],
                xd.ap()[b][:, :, t0:t0 + TBLK].transpose([1, 0, 2]),
            )
            x4 = xt[:].rearrange("p cb (tp ch) -> p cb tp ch", ch=CHUNK)

            sums = spool.tile([P, NCB, TP], F32, tag="sums")
            nc.vector.reduce_sum(sums[:], x4, axis=mybir.AxisListType.X)

            if k > 0:
                tmp = spool.tile([P, NCB], F32, tag="tmp")
                nc.vector.tensor_mul(tmp[:], qc[:], rlast[:])
                nc.vector.tensor_add(sums[:, :, 0], sums[:, :, 0], tmp[:])

            q = spool.tile([P, NCB, TP], F32, tag="q")
            nc.vector.tensor_tensor_scan(
                q[:].rearrange("p cb tp -> p (cb tp)"),
                rmask[:].rearrange("p cb tp -> p (cb tp)"),
                sums[:].rearrange("p cb tp -> p (cb tp)"),
                initial=0.0,
                op0=mybir.AluOpType.mult,
                op1=mybir.AluOpType.add,
            )
            if k < NTB - 1:
                nc.vector.tensor_copy(qc[:], q[:, :, TP - 1])

            h1 = spool.tile([P, NOC, TP], F32, tag="h1")
            for oc in range(NOC):
                acc = ps1.tile([P, TP], F32, tag="acc1")
                for cb in range(NCB):
                    nc.tensor.matmul(
                        acc[:],
                        w1[:, cb, oc * P:(oc + 1) * P],
                        q[:, cb, :],
                        start=(cb == 0),
                        stop=(cb == NCB - 1),
                    )
                nc.scalar.activation(
                    h1[:, oc, :], acc[:],
                    mybir.ActivationFunctionType.Relu,
                    bias=b1[:, oc:oc + 1],
                )

            gate = spool.tile([P, NCB, TP], F32, tag="gate")
            for ob in range(NCB):
                acc2 = ps2.tile([P, TP], F32, tag="acc2")
                for kc in range(NOC):
                    nc.tensor.matmul(
                        acc2[:],
                        w2[:, kc, ob * P:(ob + 1) * P],
                        h1[:, kc, :],
                        start=(kc == 0),
                        stop=(kc == NOC - 1),
                    )
                nc.scalar.activation(
                    gate[:, ob, :], acc2[:],
                    mybir.ActivationFunctionType.Sigmoid,
                    bias=b2[:, ob:ob + 1],
                )

            gb = gate[:].unsqueeze(3).broadcast_to([P, NCB, TP, CHUNK])
            nc.vector.tensor_mul(x4, x4, gb)
            nc.scalar.dma_start(
                outd.ap()[b][:, :, t0:t0 + TBLK].transpose([1, 0, 2]),
                xt[:],
            )


def _build_module(repeat=1):
    """Build the per-core module. repeat>1 wraps the body in a hardware loop
    that re-runs it (idempotently) for slope-based device timing."""
    nc = bacc.Bacc("TRN2", target_bir_lowering=False, debug=False,
                   num_devices=N_CORES)

    xd = nc.dram_tensor("x", [BPC, NCB, P, L], F32, kind="ExternalInput")
    w1d = nc.dram_tensor("w1t", [P, NCB, HID], F32, kind="ExternalInput")
    w2d = nc.dram_tensor("w2t", [P, NOC, C], F32, kind="ExternalInput")
    b1d = nc.dram_tensor("b1t", [P, NOC], F32, kind="ExternalInput")
    b2d = nc.dram_tensor("b2t", [P, NCB], F32, kind="ExternalInput")
    rmd = nc.dram_tensor("rmask", [P, NCB, TP], F32, kind="ExternalInput")
    rld = nc.dram_tensor("rlast", [P, NCB], F32, kind="ExternalInput")
    outd = nc.dram_tensor("out", [BPC, NCB, P, L], F32, kind="ExternalOutput")

    with tile.TileContext(nc) as tc:
        with (
            tc.tile_pool(name="const", bufs=1) as cpool,
            tc.tile_pool(name="xp", bufs=3) as xpool,
            tc.tile_pool(name="small", bufs=2) as spool,
            tc.tile_pool(name="carry", bufs=1) as carrypool,
            tc.tile_pool(name="ps1", bufs=2, space="PSUM") as ps1,
            tc.tile_pool(name="ps2", bufs=4, space="PSUM") as ps2,
        ):
            w1 = cpool.tile([P, NCB, HID], F32)
            w2 = cpool.tile([P, NOC, C], F32)
            b1 = cpool.tile([P, NOC], F32)
            b2 = cpool.tile([P, NCB], F32)
            rmask = cpool.tile([P, NCB, TP], F32)
            rlast = cpool.tile([P, NCB], F32)
            nc.gpsimd.dma_start(w1[:], w1d.ap())
            nc.gpsimd.dma_start(w2[:], w2d.ap())
            nc.gpsimd.dma_start(b1[:], b1d.ap())
            nc.gpsimd.dma_start(b2[:], b2d.ap())
            nc.gpsimd.dma_start(rmask[:], rmd.ap())
            nc.gpsimd.dma_start(rlast[:], rld.ap())

            rep = tc.For_i(0, repeat, 1) if repeat > 1 else contextlib.nullcontext()
            with rep:
                _emit_body(nc, xd, outd, w1, w2, b1, b2, rmask, rlast,
                           xpool, spool, carrypool, ps1, ps2)

    nc.compile()
    return nc


def _fake_quant(w):
    w = np.asarray(w, np.float32)
    scale = (np.max(np.abs(w), axis=1, keepdims=True).astype(np.float32)
             / np.float32(QMAX) + np.float32(EPS)).astype(np.float32)
    wq = np.clip(np.round(w / scale), -QMAX, QMAX).astype(np.float32) * scale
    return wq.astype(np.float32)


def _host_prep(w1, b1, w2, b2, ema_r):
    r = np.asarray(ema_r, np.float32)
    s = ((np.float32(1.0) - r) / np.float32(CHUNK)).astype(np.float32)

    w1s = (_fake_quant(w1) * s[None, :]).astype(np.float32)        # [HID, C]
    w1t = np.ascontiguousarray(
        w1s.T.reshape(NCB, P, HID).transpose(1, 0, 2))             # [P, NCB, HID]
    w2q = _fake_quant(w2)                                          # [C, HID]
    w2t = np.ascontiguousarray(
        w2q.T.reshape(NOC, P, C).transpose(1, 0, 2))               # [P, NOC, C]
    b1t = np.ascontiguousarray(np.asarray(b1, np.float32).reshape(NOC, P).T)
    b2t = np.ascontiguousarray(np.asarray(b2, np.float32).reshape(NCB, P).T)

    rpb = r.reshape(NCB, P).T                                      # [P, NCB]
    rmask = np.repeat(rpb[:, :, None], TP, axis=2).astype(np.float32)
    rmask[:, :, 0] = 0.0
    rlast = np.ascontiguousarray(rpb)
    return w1t, w2t, b1t, b2t, np.ascontiguousarray(rmask), rlast


def _make_in_maps(x, w1, b1, w2, b2, ema_r):
    w1t, w2t, b1t, b2t, rmask, rlast = _host_prep(w1, b1, w2, b2, ema_r)
    xh = np.asarray(x, np.float32).reshape(B, NCB, P, L)
    return [{
        "x": xh[c * BPC:(c + 1) * BPC],
        "w1t": w1t, "w2t": w2t, "b1t": b1t, "b2t": b2t,
        "rmask": rmask, "rlast": rlast,
    } for c in range(N_CORES)]


def kernel(x, w1, b1, w2, b2, ema_r):
    if "nc" not in _CACHE:
        _CACHE["nc"] = _build_module()
    nc = _CACHE["nc"]

    in_maps = _make_in_maps(x, w1, b1, w2, b2, ema_r)
    res = bass_utils.run_bass_kernel_spmd(nc, in_maps,
                                          core_ids=list(range(N_CORES)))
    out = np.empty((B, NCB, P, L), np.float32)
    for c in range(N_CORES):
        out[c * BPC:(c + 1) * BPC] = res.results[c]["out"]
    return out.reshape(B, C, L)
